# revision 12
# baseline (speedup 1.0000x reference)
"""Causal self-attention (RoPE) Trainium2 kernel.

Sharding: 2 batches x 16 heads = 32 (b,h) units over 8 cores -> each core
handles 1 batch x 4 heads. Column-parallel QKV + row-parallel output
projection; host sums the 4 partial outputs per batch.

All matmul operands are fp16 (1 cycle/row on the PE vs 2 for fp32r; fp32
PSUM accumulation). Host pre-casts x and the weight shards to fp16; the
resulting end-to-end error is ~1e-3 relative absmax.

Per-core pipeline:
  phase 1 (single pass, everything SBUF-resident):
    Q^T/K^T computed in [head_dim, t] layout. RoPE is applied with a
    duplicated-cos / signed-sin trick: the head dims are host-permuted so
    each 32-partition block holds [re pairs | im pairs] (stream_shuffle can
    only permute within 32-partition blocks), then
      out = psum*cos2 + swap16(psum*sin2s).
    V computed in [t, head_dim] layout, kept in SBUF.
  phase 2 per (head, q-tile 512): scores S^T[k,q] = (K^T chunk)^T @ Q^T,
    causal upper blocks skipped, diagonal blocks get an additive mask before
    exp (ScalarE, no max subtraction: |logits*scale| <= ~6). y^T[d,q]
    accumulates A@V in PSUM; denominator via a ones-column matmul on the
    same A tiles; normalization = fast reciprocal + gpsimd
    partition_broadcast + one multiply folded into the PSUM->SBUF copy.
  phase 3: out[q,c] accumulated over the 4 local heads from y^T chunks
    (stationary) x W_proj row shard.
"""

import sys

if "/opt/trn_rl_repo" not in sys.path:
    sys.path.insert(0, "/opt/trn_rl_repo")

import numpy as np

import concourse.bass as bass
import concourse.tile as tile
from concourse import bacc, mybir
from concourse.bass_utils import run_bass_kernel_spmd

F32 = mybir.dt.float32
F16 = mybir.dt.float16

B, T, C = 2, 2048, 2048
NH, HD = 16, 128
NHL = 4            # heads per core
D_LOC = NHL * HD   # 512 local head dims
N_CORES = 8
SCALE = 1.0 / float(np.sqrt(HD))
NEG = -30000.0     # big enough: exp((S+NEG)*SCALE) == 0 for |S| < ~1000

CC = C // 128      # 16 contraction chunks
KC = T // 128      # 16 key chunks
QT = 512           # q tile
NQT = T // QT      # 4 q tiles

_compiled = None


def _build():
    nc = bacc.Bacc("TRN2", target_bir_lowering=False, debug=False)

    xT_d = nc.dram_tensor("xT", [C, T], F16, kind="ExternalInput")
    wq_d = nc.dram_tensor("wq", [NHL, 128, CC, 128], F16, kind="ExternalInput")
    wk_d = nc.dram_tensor("wk", [NHL, 128, CC, 128], F16, kind="ExternalInput")
    wv_d = nc.dram_tensor("wv", [128, CC, D_LOC], F16, kind="ExternalInput")
    w2_d = nc.dram_tensor("w2", [128, NHL, C], F16, kind="ExternalInput")
    cos2_d = nc.dram_tensor("cos2", [128, T], F32, kind="ExternalInput")
    sin2s_d = nc.dram_tensor("sin2s", [128, T], F32, kind="ExternalInput")
    masks_d = nc.dram_tensor("masks", [128, 4, QT], F32, kind="ExternalInput")
    out_d = nc.dram_tensor("out", [T, C], F32, kind="ExternalOutput")

    swap_mask = list(range(16, 32)) + list(range(16))

    with tile.TileContext(nc) as tc:
        with tc.tile_pool(name="persist", bufs=1) as persist:
            # Q^T/K^T for the 4 heads: [:, 0:4] = Q, [:, 4:8] = K
            qkT = persist.tile([128, 8, T], F16, tag="qkT")
            # V in [t, d] layout: v_sb[:, tch, h*HD:(h+1)*HD]
            v_sb = persist.tile([128, KC, D_LOC], F16, tag="vsb")
            masks_sb = persist.tile([128, 4, QT], F32, tag="masks")
            ones_sb = persist.tile([128, 1], F16, tag="ones")
            nc.sync.dma_start(out=masks_sb, in_=masks_d.ap())
            nc.vector.memset(ones_sb, 1.0)

            # ---------------- phase 1: qkv ----------------
            with tc.tile_pool(name="px", bufs=1) as px, \
                 tc.tile_pool(name="pw", bufs=2) as pw, \
                 tc.tile_pool(name="trig", bufs=1) as ptrig, \
                 tc.tile_pool(name="rope", bufs=3) as prope, \
                 tc.tile_pool(name="psqk", bufs=4, space="PSUM") as psqk, \
                 tc.tile_pool(name="psv", bufs=3, space="PSUM") as psv:
                # first-needed DMAs first: w(jc=0), then x quarter by quarter
                w_first = pw.tile([128, CC, 128], F16, tag="w")
                nc.sync.dma_start(out=w_first, in_=wq_d.ap()[0])
                xs = px.tile([128, CC, T], F16, tag="xs")
                for tt in range(NQT):
                    for cc in range(CC):
                        nc.sync.dma_start(
                            out=xs[:, cc, tt * QT:(tt + 1) * QT],
                            in_=xT_d.ap()[cc * 128:(cc + 1) * 128,
                                          tt * QT:(tt + 1) * QT],
                        )
                cos2 = ptrig.tile([128, T], F32, tag="cos2")
                sin2s = ptrig.tile([128, T], F32, tag="sin2s")
                nc.sync.dma_start(out=cos2, in_=cos2_d.ap())
                nc.sync.dma_start(out=sin2s, in_=sin2s_d.ap())
                wv_sb = pw.tile([128, CC, D_LOC], F16, tag="wv", bufs=1)
                nc.sync.dma_start(out=wv_sb, in_=wv_d.ap())

                # Q^T and K^T (jc 0-3 -> Q head jc, 4-7 -> K head jc-4)
                for jc in range(8):
                    if jc == 0:
                        w_sb = w_first
                    else:
                        w_src = (wq_d if jc < 4 else wk_d).ap()[jc % 4]
                        w_sb = pw.tile([128, CC, 128], F16, tag="w")
                        nc.sync.dma_start(out=w_sb, in_=w_src)
                    for tt in range(NQT):
                        gt0 = tt * QT
                        ps = psqk.tile([128, QT], F32, tag="qk")
                        for cc in range(CC):
                            nc.tensor.matmul(
                                ps, w_sb[:, cc, :],
                                xs[:, cc, gt0:gt0 + QT],
                                start=(cc == 0), stop=(cc == CC - 1),
                            )
                        # rope
                        u = prope.tile([128, QT], F32, tag="u")
                        v = prope.tile([128, QT], F32, tag="v")
                        w = prope.tile([128, QT], F32, tag="w")
                        nc.vector.tensor_mul(u, ps, cos2[:, gt0:gt0 + QT])
                        nc.vector.tensor_mul(v, ps, sin2s[:, gt0:gt0 + QT])
                        nc.vector.stream_shuffle(w, v, swap_mask)
                        nc.vector.tensor_add(qkT[:, jc, gt0:gt0 + QT], u, w)

                # V
                for tch in range(KC):
                    pv = psv.tile([128, D_LOC], F32, tag="v")
                    for cc in range(CC):
                        nc.tensor.matmul(
                            pv,
                            xs[:, cc, tch * 128:(tch + 1) * 128],
                            wv_sb[:, cc, :],
                            start=(cc == 0), stop=(cc == CC - 1),
                        )
                    nc.vector.tensor_copy(v_sb[:, tch, :], pv)

            # ---------------- phase 2 + 3 ----------------
            with tc.tile_pool(name="yt", bufs=1) as pyt, \
                 tc.tile_pool(name="att", bufs=4) as patt, \
                 tc.tile_pool(name="nrm", bufs=2) as pnrm, \
                 tc.tile_pool(name="w2p", bufs=1) as pw2, \
                 tc.tile_pool(name="outp", bufs=3) as pout, \
                 tc.tile_pool(name="pst", bufs=3, space="PSUM") as pst, \
                 tc.tile_pool(name="psy", bufs=2, space="PSUM") as psy, \
                 tc.tile_pool(name="psd", bufs=2, space="PSUM") as psd:
                yT = pyt.tile([128, NHL, T], F16, tag="yT")
                w2_sb = pw2.tile([128, NHL, C], F16, tag="w2")
                nc.sync.dma_start(out=w2_sb, in_=w2_d.ap())

                for h in range(NHL):
                    for qt in range(NQT):
                        q0 = qt * QT
                        nkc = 4 * qt + 4  # valid k chunks (causal)
                        qT_ap = qkT[:, h, q0:q0 + QT]
                        yps = psy.tile([128, QT], F32, tag="y")
                        dps = psd.tile([1, QT], F32, tag="d")
                        for kc in range(nkc):
                            sps = pst.tile([128, QT], F32, tag="st")
                            nc.tensor.matmul(
                                sps, qkT[:, 4 + h, kc * 128:(kc + 1) * 128],
                                qT_ap, start=True, stop=True,
                            )
                            o = kc - 4 * qt
                            if o >= 0:
                                nc.vector.tensor_add(
                                    sps, sps, masks_sb[:, o, :])
                            a = patt.tile([128, QT], F16, tag="a")
                            nc.scalar.activation(
                                a, sps, mybir.ActivationFunctionType.Exp,
                                scale=SCALE,
                            )
                            nc.tensor.matmul(
                                yps, v_sb[:, kc, h * HD:(h + 1) * HD], a,
                                start=(kc == 0), stop=(kc == nkc - 1),
                            )
                            nc.tensor.matmul(
                                dps, ones_sb, a,
                                start=(kc == 0), stop=(kc == nkc - 1),
                            )
                        rinv = pnrm.tile([1, QT], F32, tag="rinv")
                        nc.vector.reciprocal_approx_fast(rinv, dps)
                        rb = pnrm.tile([128, QT], F32, tag="rb")
                        nc.gpsimd.partition_broadcast(rb, rinv)
                        nc.vector.tensor_mul(
                            yT[:, h, q0:q0 + QT], yps, rb)

                # ---- phase 3: out = yT^T @ w2 summed over heads ----
                for qc in range(T // 128):
                    for ct in range(C // QT):
                        ops = pst.tile([128, QT], F32, tag="st")
                        for h in range(NHL):
                            nc.tensor.matmul(
                                ops,
                                yT[:, h, qc * 128:(qc + 1) * 128],
                                w2_sb[:, h, ct * QT:(ct + 1) * QT],
                                start=(h == 0), stop=(h == NHL - 1),
                            )
                        osb = pout.tile([128, QT], F32, tag="o")
                        nc.vector.tensor_copy(osb, ops)
                        nc.sync.dma_start(
                            out=out_d.ap()[qc * 128:(qc + 1) * 128,
                                           ct * QT:(ct + 1) * QT],
                            in_=osb,
                        )

    nc.compile()
    return nc


def _prep_core_inputs(core, x16, W_attn, W_proj, cos2, sin2s, masks):
    b = core // 4
    g = core % 4
    heads = [g * NHL + i for i in range(NHL)]
    # stream_shuffle permutes within 32-partition blocks only: lay out each
    # block as [re pairs 16b..16b+15 | im pairs 16b..16b+15]
    perm = np.concatenate(
        [np.r_[2 * (16 * blk + np.arange(16)),
               2 * (16 * blk + np.arange(16)) + 1]
         for blk in range(4)]
    )

    xT = np.ascontiguousarray(x16[b].T)

    def qk_blocks(base):
        blocks = []
        for h in heads:
            blk = W_attn[:, base + h * HD: base + (h + 1) * HD][:, perm]
            blocks.append(blk.reshape(CC, 128, HD).transpose(1, 0, 2))
        return np.ascontiguousarray(np.stack(blocks, axis=0)).astype(np.float16)

    wq = qk_blocks(0)
    wk = qk_blocks(C)
    wv = np.concatenate(
        [W_attn[:, 2 * C + h * HD: 2 * C + (h + 1) * HD] for h in heads],
        axis=1,
    )  # (C, D_LOC)
    wv = np.ascontiguousarray(
        wv.reshape(CC, 128, D_LOC).transpose(1, 0, 2)).astype(np.float16)
    w2 = np.ascontiguousarray(
        np.stack([W_proj[h * HD:(h + 1) * HD, :] for h in heads], axis=0)
        .transpose(1, 0, 2)
    ).astype(np.float16)
    return {
        "xT": xT, "wq": wq, "wk": wk, "wv": wv, "w2": w2,
        "cos2": cos2, "sin2s": sin2s, "masks": masks,
    }


def _run(inputs, trace=False):
    global _compiled
    x = np.asarray(inputs["x"], dtype=np.float32)
    W_attn = np.asarray(inputs["W_attn"], dtype=np.float32)
    W_proj = np.asarray(inputs["W_proj"], dtype=np.float32)
    fc = np.asarray(inputs["freqs_cos"], dtype=np.float32)
    fs = np.asarray(inputs["freqs_sin"], dtype=np.float32)

    x16 = x.astype(np.float16)

    cosT = np.ascontiguousarray(fc.T)            # (64, T)
    sinT = np.ascontiguousarray(fs.T)
    # per 32-partition block b: partitions [0:16] carry cos/sin of pairs
    # 16b..16b+15 (re half, +sin), [16:32] the same freqs (im half, -sin)
    cos2 = np.concatenate(
        [np.concatenate([cosT[16 * blk:16 * (blk + 1)]] * 2, axis=0)
         for blk in range(4)], axis=0)           # (128, T)
    sin2s = np.concatenate(
        [np.concatenate([sinT[16 * blk:16 * (blk + 1)],
                         -sinT[16 * blk:16 * (blk + 1)]], axis=0)
         for blk in range(4)], axis=0)
    cos2 = np.ascontiguousarray(cos2)
    sin2s = np.ascontiguousarray(sin2s)

    ki = np.arange(128)[:, None]
    qi = np.arange(QT)[None, :]
    masks = np.stack(
        [np.where(128 * o + ki <= qi, 0.0, NEG).astype(np.float32)
         for o in range(4)],
        axis=1,
    )  # (128, 4, 512)
    masks = np.ascontiguousarray(masks)

    if _compiled is None:
        _compiled = _build()
    nc = _compiled

    in_maps = [
        _prep_core_inputs(c, x16, W_attn, W_proj, cos2, sin2s, masks)
        for c in range(N_CORES)
    ]
    res = run_bass_kernel_spmd(
        nc, in_maps, core_ids=list(range(N_CORES)), trace=trace)

    out = np.zeros((B, T, C), dtype=np.float32)
    for c in range(N_CORES):
        out[c // 4] += res.results[c]["out"]
    return out, res


def kernel(**inputs) -> np.ndarray:
    out, _ = _run(inputs, trace=False)
    return out


# revision 13
# speedup vs baseline: 1.0174x; 1.0174x over previous
"""Causal self-attention (RoPE) Trainium2 kernel.

Sharding: 2 batches x 16 heads = 32 (b,h) units over 8 cores -> each core
handles 1 batch x 4 heads. Column-parallel QKV + row-parallel output
projection; host sums the 4 partial outputs per batch.

All matmul operands are fp16 (1 cycle/row on the PE vs 2 for fp32r; fp32
PSUM accumulation). Host pre-casts x and the weight shards to fp16; the
resulting end-to-end error is ~1e-3 relative absmax.

Per-core pipeline:
  phase 1 (single pass, everything SBUF-resident):
    Q^T/K^T computed in [head_dim, t] layout. RoPE is applied with a
    duplicated-cos / signed-sin trick: the head dims are host-permuted so
    each 32-partition block holds [re pairs | im pairs] (stream_shuffle can
    only permute within 32-partition blocks), then
      out = psum*cos2 + swap16(psum*sin2s).
    V computed in [t, head_dim] layout, kept in SBUF.
  phase 2 per (head, q-tile 512): scores S^T[k,q] = (K^T chunk)^T @ Q^T,
    causal upper blocks skipped, diagonal blocks get an additive mask before
    exp (ScalarE, no max subtraction: |logits*scale| <= ~6). y^T[d,q]
    accumulates A@V in PSUM; denominator via a ones-column matmul on the
    same A tiles; normalization = fast reciprocal + gpsimd
    partition_broadcast + one multiply folded into the PSUM->SBUF copy.
  phase 3: out[q,c] accumulated over the 4 local heads from y^T chunks
    (stationary) x W_proj row shard.
"""

import sys

if "/opt/trn_rl_repo" not in sys.path:
    sys.path.insert(0, "/opt/trn_rl_repo")

import numpy as np

import concourse.bass as bass
import concourse.tile as tile
from concourse import bacc, mybir
from concourse.bass_utils import run_bass_kernel_spmd

F32 = mybir.dt.float32
F16 = mybir.dt.float16

B, T, C = 2, 2048, 2048
NH, HD = 16, 128
NHL = 4            # heads per core
D_LOC = NHL * HD   # 512 local head dims
N_CORES = 8
SCALE = 1.0 / float(np.sqrt(HD))
NEG = -30000.0     # big enough: exp((S+NEG)*SCALE) == 0 for |S| < ~1000

CC = C // 128      # 16 contraction chunks
KC = T // 128      # 16 key chunks
QT = 512           # q tile
NQT = T // QT      # 4 q tiles

_compiled = None


def _build():
    nc = bacc.Bacc("TRN2", target_bir_lowering=False, debug=False)

    xT_d = nc.dram_tensor("xT", [C, T], F16, kind="ExternalInput")
    wq_d = nc.dram_tensor("wq", [NHL, 128, CC, 128], F16, kind="ExternalInput")
    wk_d = nc.dram_tensor("wk", [NHL, 128, CC, 128], F16, kind="ExternalInput")
    wv_d = nc.dram_tensor("wv", [128, CC, D_LOC], F16, kind="ExternalInput")
    w2_d = nc.dram_tensor("w2", [128, NHL, C], F16, kind="ExternalInput")
    cos2_d = nc.dram_tensor("cos2", [128, T], F32, kind="ExternalInput")
    sin2s_d = nc.dram_tensor("sin2s", [128, T], F32, kind="ExternalInput")
    masks_d = nc.dram_tensor("masks", [128, 4, QT], F32, kind="ExternalInput")
    out_d = nc.dram_tensor("out", [T, C], F32, kind="ExternalOutput")

    swap_mask = list(range(16, 32)) + list(range(16))

    with tile.TileContext(nc) as tc:
        with tc.tile_pool(name="persist", bufs=1) as persist:
            # Q^T/K^T for the 4 heads: [:, 0:4] = Q, [:, 4:8] = K
            qkT = persist.tile([128, 8, T], F16, tag="qkT")
            # V in [t, d] layout: v_sb[:, tch, h*HD:(h+1)*HD]
            v_sb = persist.tile([128, KC, D_LOC], F16, tag="vsb")
            masks_sb = persist.tile([128, 4, QT], F32, tag="masks")
            ones_sb = persist.tile([128, 1], F16, tag="ones")
            nc.sync.dma_start(out=masks_sb, in_=masks_d.ap())
            nc.vector.memset(ones_sb, 1.0)

            # ---------------- phase 1: qkv ----------------
            with tc.tile_pool(name="px", bufs=1) as px, \
                 tc.tile_pool(name="pw", bufs=2) as pw, \
                 tc.tile_pool(name="trig", bufs=1) as ptrig, \
                 tc.tile_pool(name="rope", bufs=3) as prope, \
                 tc.tile_pool(name="psqk", bufs=4, space="PSUM") as psqk, \
                 tc.tile_pool(name="psv", bufs=3, space="PSUM") as psv:
                # first-needed DMAs first: w(jc=0), then x quarter by quarter
                w_first = pw.tile([128, CC, 128], F16, tag="w")
                nc.sync.dma_start(out=w_first, in_=wq_d.ap()[0])
                xs = px.tile([128, CC, T], F16, tag="xs")
                cos2 = ptrig.tile([128, T], F32, tag="cos2")
                sin2s = ptrig.tile([128, T], F32, tag="sin2s")
                for cc in range(CC):
                    nc.sync.dma_start(
                        out=xs[:, cc, 0:QT],
                        in_=xT_d.ap()[cc * 128:(cc + 1) * 128, 0:QT],
                    )
                nc.sync.dma_start(out=cos2, in_=cos2_d.ap())
                nc.sync.dma_start(out=sin2s, in_=sin2s_d.ap())
                for tt in range(1, NQT):
                    for cc in range(CC):
                        nc.sync.dma_start(
                            out=xs[:, cc, tt * QT:(tt + 1) * QT],
                            in_=xT_d.ap()[cc * 128:(cc + 1) * 128,
                                          tt * QT:(tt + 1) * QT],
                        )
                wv_sb = pw.tile([128, CC, D_LOC], F16, tag="wv", bufs=1)
                nc.sync.dma_start(out=wv_sb, in_=wv_d.ap())

                # Q^T and K^T (jc 0-3 -> Q head jc, 4-7 -> K head jc-4)
                for jc in range(8):
                    if jc == 0:
                        w_sb = w_first
                    else:
                        w_src = (wq_d if jc < 4 else wk_d).ap()[jc % 4]
                        w_sb = pw.tile([128, CC, 128], F16, tag="w")
                        nc.sync.dma_start(out=w_sb, in_=w_src)
                    for tt in range(NQT):
                        gt0 = tt * QT
                        ps = psqk.tile([128, QT], F32, tag="qk")
                        for cc in range(CC):
                            nc.tensor.matmul(
                                ps, w_sb[:, cc, :],
                                xs[:, cc, gt0:gt0 + QT],
                                start=(cc == 0), stop=(cc == CC - 1),
                            )
                        # rope
                        u = prope.tile([128, QT], F32, tag="u")
                        v = prope.tile([128, QT], F32, tag="v")
                        w = prope.tile([128, QT], F32, tag="w")
                        nc.vector.tensor_mul(u, ps, cos2[:, gt0:gt0 + QT])
                        nc.vector.tensor_mul(v, ps, sin2s[:, gt0:gt0 + QT])
                        nc.vector.stream_shuffle(w, v, swap_mask)
                        nc.vector.tensor_add(qkT[:, jc, gt0:gt0 + QT], u, w)

                # V
                for tch in range(KC):
                    pv = psv.tile([128, D_LOC], F32, tag="v")
                    for cc in range(CC):
                        nc.tensor.matmul(
                            pv,
                            xs[:, cc, tch * 128:(tch + 1) * 128],
                            wv_sb[:, cc, :],
                            start=(cc == 0), stop=(cc == CC - 1),
                        )
                    nc.vector.tensor_copy(v_sb[:, tch, :], pv)

            # ---------------- phase 2 + 3 ----------------
            with tc.tile_pool(name="yt", bufs=1) as pyt, \
                 tc.tile_pool(name="att", bufs=4) as patt, \
                 tc.tile_pool(name="nrm", bufs=2) as pnrm, \
                 tc.tile_pool(name="w2p", bufs=1) as pw2, \
                 tc.tile_pool(name="outp", bufs=3) as pout, \
                 tc.tile_pool(name="pst", bufs=3, space="PSUM") as pst, \
                 tc.tile_pool(name="psy", bufs=2, space="PSUM") as psy, \
                 tc.tile_pool(name="psd", bufs=2, space="PSUM") as psd:
                yT = pyt.tile([128, NHL, T], F16, tag="yT")
                w2_sb = pw2.tile([128, NHL, C], F16, tag="w2")
                nc.sync.dma_start(out=w2_sb, in_=w2_d.ap())

                for h in range(NHL):
                    for qt in range(NQT):
                        q0 = qt * QT
                        nkc = 4 * qt + 4  # valid k chunks (causal)
                        qT_ap = qkT[:, h, q0:q0 + QT]
                        yps = psy.tile([128, QT], F32, tag="y")
                        dps = psd.tile([1, QT], F32, tag="d")
                        for kc in range(nkc):
                            sps = pst.tile([128, QT], F32, tag="st")
                            nc.tensor.matmul(
                                sps, qkT[:, 4 + h, kc * 128:(kc + 1) * 128],
                                qT_ap, start=True, stop=True,
                            )
                            o = kc - 4 * qt
                            if o >= 0:
                                nc.vector.tensor_add(
                                    sps, sps, masks_sb[:, o, :])
                            a = patt.tile([128, QT], F16, tag="a")
                            nc.scalar.activation(
                                a, sps, mybir.ActivationFunctionType.Exp,
                                scale=SCALE,
                            )
                            nc.tensor.matmul(
                                yps, v_sb[:, kc, h * HD:(h + 1) * HD], a,
                                start=(kc == 0), stop=(kc == nkc - 1),
                            )
                            nc.tensor.matmul(
                                dps, ones_sb, a,
                                start=(kc == 0), stop=(kc == nkc - 1),
                            )
                        rinv = pnrm.tile([1, QT], F32, tag="rinv")
                        nc.vector.reciprocal_approx_fast(rinv, dps)
                        rb = pnrm.tile([128, QT], F32, tag="rb")
                        nc.gpsimd.partition_broadcast(rb, rinv)
                        nc.vector.tensor_mul(
                            yT[:, h, q0:q0 + QT], yps, rb)

                # ---- phase 3: out = yT^T @ w2 summed over heads ----
                for qc in range(T // 128):
                    for ct in range(C // QT):
                        ops = pst.tile([128, QT], F32, tag="st")
                        for h in range(NHL):
                            nc.tensor.matmul(
                                ops,
                                yT[:, h, qc * 128:(qc + 1) * 128],
                                w2_sb[:, h, ct * QT:(ct + 1) * QT],
                                start=(h == 0), stop=(h == NHL - 1),
                            )
                        osb = pout.tile([128, QT], F32, tag="o")
                        nc.vector.tensor_copy(osb, ops)
                        nc.sync.dma_start(
                            out=out_d.ap()[qc * 128:(qc + 1) * 128,
                                           ct * QT:(ct + 1) * QT],
                            in_=osb,
                        )

    nc.compile()
    return nc


def _prep_core_inputs(core, x16, W_attn, W_proj, cos2, sin2s, masks):
    b = core // 4
    g = core % 4
    heads = [g * NHL + i for i in range(NHL)]
    # stream_shuffle permutes within 32-partition blocks only: lay out each
    # block as [re pairs 16b..16b+15 | im pairs 16b..16b+15]
    perm = np.concatenate(
        [np.r_[2 * (16 * blk + np.arange(16)),
               2 * (16 * blk + np.arange(16)) + 1]
         for blk in range(4)]
    )

    xT = np.ascontiguousarray(x16[b].T)

    def qk_blocks(base):
        blocks = []
        for h in heads:
            blk = W_attn[:, base + h * HD: base + (h + 1) * HD][:, perm]
            blocks.append(blk.reshape(CC, 128, HD).transpose(1, 0, 2))
        return np.ascontiguousarray(np.stack(blocks, axis=0)).astype(np.float16)

    wq = qk_blocks(0)
    wk = qk_blocks(C)
    wv = np.concatenate(
        [W_attn[:, 2 * C + h * HD: 2 * C + (h + 1) * HD] for h in heads],
        axis=1,
    )  # (C, D_LOC)
    wv = np.ascontiguousarray(
        wv.reshape(CC, 128, D_LOC).transpose(1, 0, 2)).astype(np.float16)
    w2 = np.ascontiguousarray(
        np.stack([W_proj[h * HD:(h + 1) * HD, :] for h in heads], axis=0)
        .transpose(1, 0, 2)
    ).astype(np.float16)
    return {
        "xT": xT, "wq": wq, "wk": wk, "wv": wv, "w2": w2,
        "cos2": cos2, "sin2s": sin2s, "masks": masks,
    }


def _run(inputs, trace=False):
    global _compiled
    x = np.asarray(inputs["x"], dtype=np.float32)
    W_attn = np.asarray(inputs["W_attn"], dtype=np.float32)
    W_proj = np.asarray(inputs["W_proj"], dtype=np.float32)
    fc = np.asarray(inputs["freqs_cos"], dtype=np.float32)
    fs = np.asarray(inputs["freqs_sin"], dtype=np.float32)

    x16 = x.astype(np.float16)

    cosT = np.ascontiguousarray(fc.T)            # (64, T)
    sinT = np.ascontiguousarray(fs.T)
    # per 32-partition block b: partitions [0:16] carry cos/sin of pairs
    # 16b..16b+15 (re half, +sin), [16:32] the same freqs (im half, -sin)
    cos2 = np.concatenate(
        [np.concatenate([cosT[16 * blk:16 * (blk + 1)]] * 2, axis=0)
         for blk in range(4)], axis=0)           # (128, T)
    sin2s = np.concatenate(
        [np.concatenate([sinT[16 * blk:16 * (blk + 1)],
                         -sinT[16 * blk:16 * (blk + 1)]], axis=0)
         for blk in range(4)], axis=0)
    cos2 = np.ascontiguousarray(cos2)
    sin2s = np.ascontiguousarray(sin2s)

    ki = np.arange(128)[:, None]
    qi = np.arange(QT)[None, :]
    masks = np.stack(
        [np.where(128 * o + ki <= qi, 0.0, NEG).astype(np.float32)
         for o in range(4)],
        axis=1,
    )  # (128, 4, 512)
    masks = np.ascontiguousarray(masks)

    if _compiled is None:
        _compiled = _build()
    nc = _compiled

    in_maps = [
        _prep_core_inputs(c, x16, W_attn, W_proj, cos2, sin2s, masks)
        for c in range(N_CORES)
    ]
    res = run_bass_kernel_spmd(
        nc, in_maps, core_ids=list(range(N_CORES)), trace=trace)

    out = np.zeros((B, T, C), dtype=np.float32)
    for c in range(N_CORES):
        out[c // 4] += res.results[c]["out"]
    return out, res


def kernel(**inputs) -> np.ndarray:
    out, _ = _run(inputs, trace=False)
    return out


# revision 15
# speedup vs baseline: 1.0229x; 1.0054x over previous
"""Causal self-attention (RoPE) Trainium2 kernel.

Sharding: 2 batches x 16 heads = 32 (b,h) units over 8 cores -> each core
handles 1 batch x 4 heads. Column-parallel QKV + row-parallel output
projection; host sums the 4 partial outputs per batch.

All matmul operands are fp16 (1 cycle/row on the PE vs 2 for fp32r; fp32
PSUM accumulation). Host pre-casts x and the weight shards to fp16; the
resulting end-to-end error is ~1e-3 relative absmax.

Per-core pipeline:
  phase 1 (single pass, everything SBUF-resident):
    Q^T/K^T computed in [head_dim, t] layout. RoPE is applied with a
    duplicated-cos / signed-sin trick: the head dims are host-permuted so
    each 32-partition block holds [re pairs | im pairs] (stream_shuffle can
    only permute within 32-partition blocks), then
      out = psum*cos2 + swap16(psum*sin2s).
    V computed in [t, head_dim] layout, kept in SBUF.
  phase 2 per (head, q-tile 512): scores S^T[k,q] = (K^T chunk)^T @ Q^T,
    causal upper blocks skipped, diagonal blocks get an additive mask before
    exp (ScalarE, no max subtraction: |logits*scale| <= ~6). y^T[d,q]
    accumulates A@V in PSUM; denominator via a ones-column matmul on the
    same A tiles; normalization = fast reciprocal + gpsimd
    partition_broadcast + one multiply folded into the PSUM->SBUF copy.
  phase 3: out[q,c] accumulated over the 4 local heads from y^T chunks
    (stationary) x W_proj row shard.
"""

import sys

if "/opt/trn_rl_repo" not in sys.path:
    sys.path.insert(0, "/opt/trn_rl_repo")

import numpy as np

import concourse.bass as bass
import concourse.tile as tile
from concourse import bacc, mybir
from concourse.bass_utils import run_bass_kernel_spmd

F32 = mybir.dt.float32
F16 = mybir.dt.float16

B, T, C = 2, 2048, 2048
NH, HD = 16, 128
NHL = 4            # heads per core
D_LOC = NHL * HD   # 512 local head dims
N_CORES = 8
SCALE = 1.0 / float(np.sqrt(HD))
NEG = -30000.0     # big enough: exp((S+NEG)*SCALE) == 0 for |S| < ~1000

CC = C // 128      # 16 contraction chunks
KC = T // 128      # 16 key chunks
QT = 512           # q tile
NQT = T // QT      # 4 q tiles

_compiled = None


def _build():
    nc = bacc.Bacc("TRN2", target_bir_lowering=False, debug=False)

    xT_d = nc.dram_tensor("xT", [C, T], F16, kind="ExternalInput")
    wq_d = nc.dram_tensor("wq", [NHL, 128, CC, 128], F16, kind="ExternalInput")
    wk_d = nc.dram_tensor("wk", [NHL, 128, CC, 128], F16, kind="ExternalInput")
    wv_d = nc.dram_tensor("wv", [128, CC, D_LOC], F16, kind="ExternalInput")
    w2_d = nc.dram_tensor("w2", [128, NHL, C], F16, kind="ExternalInput")
    cos2_d = nc.dram_tensor("cos2", [128, T], F32, kind="ExternalInput")
    sin2s_d = nc.dram_tensor("sin2s", [128, T], F32, kind="ExternalInput")
    masks_d = nc.dram_tensor("masks", [128, 4, QT], F32, kind="ExternalInput")
    out_d = nc.dram_tensor("out", [T, C], F32, kind="ExternalOutput")

    swap_mask = list(range(16, 32)) + list(range(16))

    with tile.TileContext(nc) as tc:
        with tc.tile_pool(name="persist", bufs=1) as persist:
            # Q^T/K^T for the 4 heads: [:, 0:4] = Q, [:, 4:8] = K
            qkT = persist.tile([128, 8, T], F16, tag="qkT")
            # V in [t, d] layout: v_sb[:, tch, h*HD:(h+1)*HD]
            v_sb = persist.tile([128, KC, D_LOC], F16, tag="vsb")
            masks_sb = persist.tile([128, 4, QT], F32, tag="masks")
            ones_sb = persist.tile([128, 1], F16, tag="ones")
            nc.sync.dma_start(out=masks_sb, in_=masks_d.ap())
            nc.vector.memset(ones_sb, 1.0)

            # ---------------- phase 1: qkv ----------------
            with tc.tile_pool(name="px", bufs=1) as px, \
                 tc.tile_pool(name="pw", bufs=8) as pw, \
                 tc.tile_pool(name="trig", bufs=1) as ptrig, \
                 tc.tile_pool(name="rope", bufs=3) as prope, \
                 tc.tile_pool(name="psqk", bufs=4, space="PSUM") as psqk, \
                 tc.tile_pool(name="psv", bufs=3, space="PSUM") as psv:
                # first-needed DMAs first: w(jc=0), then x quarter by quarter
                w_first = pw.tile([128, CC, 128], F16, tag="w")
                nc.sync.dma_start(out=w_first, in_=wq_d.ap()[0])
                xs = px.tile([128, CC, T], F16, tag="xs")
                cos2 = ptrig.tile([128, T], F32, tag="cos2")
                sin2s = ptrig.tile([128, T], F32, tag="sin2s")
                for cc in range(CC):
                    nc.sync.dma_start(
                        out=xs[:, cc, 0:QT],
                        in_=xT_d.ap()[cc * 128:(cc + 1) * 128, 0:QT],
                    )
                nc.sync.dma_start(out=cos2, in_=cos2_d.ap())
                nc.sync.dma_start(out=sin2s, in_=sin2s_d.ap())
                for tt in range(1, NQT):
                    for cc in range(CC):
                        nc.sync.dma_start(
                            out=xs[:, cc, tt * QT:(tt + 1) * QT],
                            in_=xT_d.ap()[cc * 128:(cc + 1) * 128,
                                          tt * QT:(tt + 1) * QT],
                        )
                w_tiles = [w_first]
                for jc in range(1, 8):
                    w_src = (wq_d if jc < 4 else wk_d).ap()[jc % 4]
                    w_sb = pw.tile([128, CC, 128], F16, tag="w",
                                   name=f"w_sb{jc}")
                    nc.sync.dma_start(out=w_sb, in_=w_src)
                    w_tiles.append(w_sb)
                wv_sb = pw.tile([128, CC, D_LOC], F16, tag="wv", bufs=1)
                nc.sync.dma_start(out=wv_sb, in_=wv_d.ap())

                # Q^T and K^T (jc 0-3 -> Q head jc, 4-7 -> K head jc-4)
                for jc in range(8):
                    w_sb = w_tiles[jc]
                    for tt in range(NQT):
                        gt0 = tt * QT
                        ps = psqk.tile([128, QT], F32, tag="qk")
                        for cc in range(CC):
                            nc.tensor.matmul(
                                ps, w_sb[:, cc, :],
                                xs[:, cc, gt0:gt0 + QT],
                                start=(cc == 0), stop=(cc == CC - 1),
                            )
                        # rope
                        u = prope.tile([128, QT], F32, tag="u")
                        v = prope.tile([128, QT], F32, tag="v")
                        w = prope.tile([128, QT], F32, tag="w")
                        nc.vector.tensor_mul(u, ps, cos2[:, gt0:gt0 + QT])
                        nc.vector.tensor_mul(v, ps, sin2s[:, gt0:gt0 + QT])
                        nc.vector.stream_shuffle(w, v, swap_mask)
                        nc.vector.tensor_add(qkT[:, jc, gt0:gt0 + QT], u, w)

                # V
                for tch in range(KC):
                    pv = psv.tile([128, D_LOC], F32, tag="v")
                    for cc in range(CC):
                        nc.tensor.matmul(
                            pv,
                            xs[:, cc, tch * 128:(tch + 1) * 128],
                            wv_sb[:, cc, :],
                            start=(cc == 0), stop=(cc == CC - 1),
                        )
                    nc.scalar.copy(v_sb[:, tch, :], pv)

            # ---------------- phase 2 + 3 ----------------
            with tc.tile_pool(name="yt", bufs=1) as pyt, \
                 tc.tile_pool(name="att", bufs=4) as patt, \
                 tc.tile_pool(name="nrm", bufs=2) as pnrm, \
                 tc.tile_pool(name="w2p", bufs=1) as pw2, \
                 tc.tile_pool(name="outp", bufs=3) as pout, \
                 tc.tile_pool(name="pst", bufs=3, space="PSUM") as pst, \
                 tc.tile_pool(name="psy", bufs=2, space="PSUM") as psy, \
                 tc.tile_pool(name="psd", bufs=2, space="PSUM") as psd:
                yT = pyt.tile([128, NHL, T], F16, tag="yT")
                w2_sb = pw2.tile([128, NHL, C], F16, tag="w2")
                nc.sync.dma_start(out=w2_sb, in_=w2_d.ap())

                for h in range(NHL):
                    for qt in range(NQT):
                        q0 = qt * QT
                        nkc = 4 * qt + 4  # valid k chunks (causal)
                        qT_ap = qkT[:, h, q0:q0 + QT]
                        yps = psy.tile([128, QT], F32, tag="y")
                        dps = psd.tile([1, QT], F32, tag="d")
                        for kc in range(nkc):
                            sps = pst.tile([128, QT], F32, tag="st")
                            nc.tensor.matmul(
                                sps, qkT[:, 4 + h, kc * 128:(kc + 1) * 128],
                                qT_ap, start=True, stop=True,
                            )
                            o = kc - 4 * qt
                            if o >= 0:
                                nc.vector.tensor_add(
                                    sps, sps, masks_sb[:, o, :])
                            a = patt.tile([128, QT], F16, tag="a")
                            nc.scalar.activation(
                                a, sps, mybir.ActivationFunctionType.Exp,
                                scale=SCALE,
                            )
                            nc.tensor.matmul(
                                yps, v_sb[:, kc, h * HD:(h + 1) * HD], a,
                                start=(kc == 0), stop=(kc == nkc - 1),
                            )
                            nc.tensor.matmul(
                                dps, ones_sb, a,
                                start=(kc == 0), stop=(kc == nkc - 1),
                            )
                        rinv = pnrm.tile([1, QT], F32, tag="rinv")
                        nc.vector.reciprocal_approx_fast(rinv, dps)
                        rb = pnrm.tile([128, QT], F32, tag="rb")
                        nc.gpsimd.partition_broadcast(rb, rinv)
                        nc.vector.tensor_mul(
                            yT[:, h, q0:q0 + QT], yps, rb)

                # ---- phase 3: out = yT^T @ w2 summed over heads ----
                for qc in range(T // 128):
                    for ct in range(C // QT):
                        ops = pst.tile([128, QT], F32, tag="st")
                        for h in range(NHL):
                            nc.tensor.matmul(
                                ops,
                                yT[:, h, qc * 128:(qc + 1) * 128],
                                w2_sb[:, h, ct * QT:(ct + 1) * QT],
                                start=(h == 0), stop=(h == NHL - 1),
                            )
                        osb = pout.tile([128, QT], F32, tag="o")
                        nc.vector.tensor_copy(osb, ops)
                        nc.sync.dma_start(
                            out=out_d.ap()[qc * 128:(qc + 1) * 128,
                                           ct * QT:(ct + 1) * QT],
                            in_=osb,
                        )

    nc.compile()
    return nc


def _prep_core_inputs(core, x16, W_attn, W_proj, cos2, sin2s, masks):
    b = core // 4
    g = core % 4
    heads = [g * NHL + i for i in range(NHL)]
    # stream_shuffle permutes within 32-partition blocks only: lay out each
    # block as [re pairs 16b..16b+15 | im pairs 16b..16b+15]
    perm = np.concatenate(
        [np.r_[2 * (16 * blk + np.arange(16)),
               2 * (16 * blk + np.arange(16)) + 1]
         for blk in range(4)]
    )

    xT = np.ascontiguousarray(x16[b].T)

    def qk_blocks(base):
        blocks = []
        for h in heads:
            blk = W_attn[:, base + h * HD: base + (h + 1) * HD][:, perm]
            blocks.append(blk.reshape(CC, 128, HD).transpose(1, 0, 2))
        return np.ascontiguousarray(np.stack(blocks, axis=0)).astype(np.float16)

    wq = qk_blocks(0)
    wk = qk_blocks(C)
    wv = np.concatenate(
        [W_attn[:, 2 * C + h * HD: 2 * C + (h + 1) * HD] for h in heads],
        axis=1,
    )  # (C, D_LOC)
    wv = np.ascontiguousarray(
        wv.reshape(CC, 128, D_LOC).transpose(1, 0, 2)).astype(np.float16)
    w2 = np.ascontiguousarray(
        np.stack([W_proj[h * HD:(h + 1) * HD, :] for h in heads], axis=0)
        .transpose(1, 0, 2)
    ).astype(np.float16)
    return {
        "xT": xT, "wq": wq, "wk": wk, "wv": wv, "w2": w2,
        "cos2": cos2, "sin2s": sin2s, "masks": masks,
    }


def _run(inputs, trace=False):
    global _compiled
    x = np.asarray(inputs["x"], dtype=np.float32)
    W_attn = np.asarray(inputs["W_attn"], dtype=np.float32)
    W_proj = np.asarray(inputs["W_proj"], dtype=np.float32)
    fc = np.asarray(inputs["freqs_cos"], dtype=np.float32)
    fs = np.asarray(inputs["freqs_sin"], dtype=np.float32)

    x16 = x.astype(np.float16)

    cosT = np.ascontiguousarray(fc.T)            # (64, T)
    sinT = np.ascontiguousarray(fs.T)
    # per 32-partition block b: partitions [0:16] carry cos/sin of pairs
    # 16b..16b+15 (re half, +sin), [16:32] the same freqs (im half, -sin)
    cos2 = np.concatenate(
        [np.concatenate([cosT[16 * blk:16 * (blk + 1)]] * 2, axis=0)
         for blk in range(4)], axis=0)           # (128, T)
    sin2s = np.concatenate(
        [np.concatenate([sinT[16 * blk:16 * (blk + 1)],
                         -sinT[16 * blk:16 * (blk + 1)]], axis=0)
         for blk in range(4)], axis=0)
    cos2 = np.ascontiguousarray(cos2)
    sin2s = np.ascontiguousarray(sin2s)

    ki = np.arange(128)[:, None]
    qi = np.arange(QT)[None, :]
    masks = np.stack(
        [np.where(128 * o + ki <= qi, 0.0, NEG).astype(np.float32)
         for o in range(4)],
        axis=1,
    )  # (128, 4, 512)
    masks = np.ascontiguousarray(masks)

    if _compiled is None:
        _compiled = _build()
    nc = _compiled

    in_maps = [
        _prep_core_inputs(c, x16, W_attn, W_proj, cos2, sin2s, masks)
        for c in range(N_CORES)
    ]
    res = run_bass_kernel_spmd(
        nc, in_maps, core_ids=list(range(N_CORES)), trace=trace)

    out = np.zeros((B, T, C), dtype=np.float32)
    for c in range(N_CORES):
        out[c // 4] += res.results[c]["out"]
    return out, res


def kernel(**inputs) -> np.ndarray:
    out, _ = _run(inputs, trace=False)
    return out
